# revision 1
# baseline (speedup 1.0000x reference)
"""Trainium2 Bass kernel for nn_DictMoEDirect (moe_routing), v2.

Reference computation (shapes hardcoded):
  x = hidden_states.transpose(1,0,2)              # [B,S,H]
  g = mean_s(relu(x@gW1.T + gb1) @ gW2.T + gb2)   # [B,E]
  W1_b = sum_e g[b,e] eW1[e]; b1_b = g[b]@eb1     # per-sample merged MLP
  W2_b = sum_e g[b,e] eW2[e]; b2_b = g[b]@eb2
  y = relu(x@W1_b.T + b1_b) @ W2_b.T + b2_b       # [B,S,H]
  return y.transpose(1,0,2)                       # [S,B,H]

Distribution over 8 NeuronCores (v2):
  - Gate: data-parallel (core b computes g[b]), tiny AllGather of g.
  - FFN: tensor-parallel over DFF (core j owns a 512-wide slice).  Layer-2
    partial products are reduce-scattered in 4 bf16 chunks (per H-half x
    m-pair) so all but the last chunk overlap compute.
  - All bulk data is bf16 (same PE throughput as f32r, half the DMA).
  - The per-sample weight merges (W_b = sum_e g[b,e] E_e), which cost 2x the
    GEMM cycles if done on the PE alone (diag-matmul trick), are split across
    three engines, per-sample granularity, tunable below:
      "pe":   diag-matmul accumulation in PSUM (e-outer so LDWEIGHTS hides)
      "tree": ScalarE scaled-copies (Act) + VectorE add tree
      "pure": VectorE scaled-copies (4x mode) + add tree
  - y1 stays resident in SBUF (no DRAM round trip).

kernel(**inputs) takes full unsharded inputs, shards/transposes on the host,
runs the SPMD kernel, reassembles the full [S,B,H] output.
"""

import numpy as np

import concourse.bass as bass  # noqa: F401
import concourse.mybir as mybir
from concourse import bacc
from concourse.tile import TileContext
from concourse.masks import make_identity

H = 1024
DFF = 4096
E = 8
B = 8
S = 512
NC = 8
DSL = DFF // NC  # 512, per-core DFF slice
P = 128
F32 = mybir.dt.float32
BF16 = mybir.dt.bfloat16
AF = mybir.ActivationFunctionType

# merge-engine assignment per sample index, per layer (tunable).
# Gemm consumption order puts "pe" samples first (their merges finish first),
# then "pure" (DVE runs ahead), then "tree" (Act is the serial bottleneck).
L1_MODE = {0: "tree", 1: "tree", 2: "tree", 3: "pure", 4: "pure",
           5: "pe", 6: "pe", 7: "pe"}
L2_MODE = {0: "tree", 1: "tree", 2: "tree", 3: "pure", 4: "pure",
           5: "pe", 6: "pe", 7: "pe"}
# Emission == consumption order (avoids ring-buffer ordering cycles).
# PE-merged samples first, then tree/pure interleaved so Act starts early
# while DVE alternates between its own merges and tree adds.
# The tile pool serializes each tag's allocations' FIRST WRITES in
# allocation order, so pe samples (first write = PSUM copy, trailing the
# PE merge cadence) get their own tag "w1p" (all fresh buffers - a ring
# wait on a later gemm would deadlock against the mm-PSUM ring), while
# tree/pure samples (first write = an add right after g lands) share
# ring "w1t" whose reuse waits only on the earliest gemms.
ALLOC1_ORDER = [5, 6, 7, 0, 3, 1, 4, 2]
GEMM1_ORDER = [5, 6, 7, 0, 3, 1, 4, 2]
MERGE2_ORDER = [5, 6, 7, 0, 3, 1, 4, 2]
GEMM2_ORDER = [5, 6, 7, 3, 4, 0, 1, 2]  # tree-merged last (ready latest)


def build_module(time_loop=0, time_phase=0):
    """time_loop=R wraps the FFN phases (not gate/collectives) in an
    on-device For loop for timing runs; outputs are then meaningless."""
    nc = bacc.Bacc()

    # ---- I/O (all per-core) ----
    xt_all = nc.declare_dram_parameter("xt_all", [B, H, S], BF16, isOutput=False)
    xt_own = nc.declare_dram_parameter("xt_own", [H, S], BF16, isOutput=False)
    gw1t = nc.declare_dram_parameter("gw1t", [H, H], BF16, isOutput=False)
    gb1t = nc.declare_dram_parameter("gb1t", [P, 8], F32, isOutput=False)
    gw2t = nc.declare_dram_parameter("gw2t", [H, E], BF16, isOutput=False)
    gb2 = nc.declare_dram_parameter("gb2", [E], F32, isOutput=False)
    # ew1d[p, k, e, d] = eW1[e, d, k*128+p]   (i-part, i-tile, expert, out)
    ew1d = nc.declare_dram_parameter("ew1d", [P, 8 * E * DSL], BF16, isOutput=False)
    # ew2d[h, p, e, kt, c] = eW2[e, h*512+c, kt*128+p]
    ew2d = nc.declare_dram_parameter("ew2d", [2, P, E * 4 * 512], BF16, isOutput=False)
    eb1s = nc.declare_dram_parameter("eb1s", [E, DSL], BF16, isOutput=False)
    eb2 = nc.declare_dram_parameter("eb2", [E, H], F32, isOutput=False)
    y_out = nc.declare_dram_parameter("y2t", [H, S], BF16, isOutput=True)

    # ---- internal DRAM ----
    ag_in = nc.dram_tensor("ag_in", [E], F32)
    ag_out = nc.dram_tensor("ag_out", [NC * E], F32, addr_space="Shared")
    rs_in = nc.dram_tensor("rs_in", [2, 2, B, 2, P, S], BF16)
    rs_out = nc.dram_tensor("rs_out", [2, 2, 2 * P, S], BF16)
    groups = [list(range(NC))]

    with TileContext(nc) as tc:
        with (
            tc.tile_pool(name="main", bufs=1) as pool,
            tc.tile_pool(name="psum", bufs=1, space="PSUM") as pp,
        ):
            # =================== gate (own sample) ===================
            xo = pool.tile([P, 8 * S], BF16, tag="x8", bufs=3)
            nc.sync.dma_start(
                xo[:].rearrange("p (k s) -> p k s", k=8),
                xt_own.rearrange("(k p) s -> p k s", p=P),
            )
            gb1_sb = pool.tile([P, 8], F32, tag="gb1")
            nc.sync.dma_start(gb1_sb[:], gb1t[:])
            gw1_sb = pool.tile([P, 16384], BF16, tag="B32", bufs=3)
            gw1v = gw1_sb[:].rearrange("p (k o) -> p k o", k=8)[:, :, :H]
            for k in range(8):
                nc.sync.dma_start(gw1v[:, k], gw1t[k * P : (k + 1) * P, :])
            h1 = pool.tile([P, 8 * S], BF16, tag="x8", bufs=3)
            h1v = h1[:].rearrange("p (m s) -> p m s", m=8)
            for m in range(8):
                ps = pp.tile([P, S], F32, tag="out", bufs=3)
                for k in range(8):
                    nc.tensor.matmul(
                        ps[:],
                        gw1v[:, k, m * P : (m + 1) * P],
                        xo[:, k * S : (k + 1) * S],
                        start=(k == 0),
                        stop=(k == 7),
                    )
                nc.scalar.activation(
                    h1v[:, m], ps[:], AF.Relu, bias=gb1_sb[:, m : m + 1]
                )
            gw2_sb = pool.tile([P, 8 * E], BF16, tag="gw2")
            for k in range(8):
                nc.sync.dma_start(
                    gw2_sb[:, k * E : (k + 1) * E], gw2t[k * P : (k + 1) * P, :]
                )
            ps_g = pp.tile([E, S], F32, tag="tiny")
            for k in range(8):
                nc.tensor.matmul(
                    ps_g[:],
                    gw2_sb[:, k * E : (k + 1) * E],
                    h1v[:, k],
                    start=(k == 0),
                    stop=(k == 7),
                )
            gsum = pool.tile([E, 1], F32, tag="gsum")
            nc.vector.reduce_sum(gsum[:], ps_g[:], axis=mybir.AxisListType.X)
            gb2_sb = pool.tile([E, 1], F32, tag="gb2")
            nc.gpsimd.dma_start(gb2_sb[:], gb2[:, None])
            gmean = pool.tile([E, 1], F32, tag="gmean")
            nc.vector.tensor_scalar_mul(gmean[:], gsum[:], 1.0 / S)
            gown = pool.tile([E, 1], F32, tag="gown")
            nc.vector.tensor_add(gown[:], gmean[:], gb2_sb[:])
            nc.sync.dma_start(ag_in[:], gown[:, 0])

            nc.gpsimd.collective_compute(
                "AllGather",
                mybir.AluOpType.bypass,
                ins=[ag_in[:]],
                outs=[ag_out[:]],
                replica_groups=groups,
            )

            # g broadcast across partitions [P, B*E] f32
            g_bc = pool.tile([P, NC * E], F32, tag="gbc")
            nc.gpsimd.dma_start(
                g_bc[:], ag_out.ap()[None, :].broadcast_to([P, NC * E])
            )
            gT_f = pool.tile([E, B], F32, tag="gTf")
            nc.gpsimd.dma_start(gT_f[:], ag_out.rearrange("(b e) -> e b", e=E))
            gT_r = pool.tile([E, B], BF16, tag="gT")
            nc.vector.tensor_copy(gT_r[:], gT_f[:])

            # identity (bf16) for the scaled-diag merge trick
            eye = pool.tile([P, P], F32, tag="eye")
            make_identity(nc, eye[:])
            eye_bf = pool.tile([P, P], BF16, tag="eyeb")
            nc.vector.tensor_copy(eye_bf[:], eye[:])

            # ---- merged per-sample biases ----
            # b1t[:, mt*8+b] = (g[b] @ eb1s)[mt-tile]      (full value)
            # b2t[:, m*8+b]  = (g[b] @ eb2)[m-tile] / 8    (1/8: summed by RS)
            eb1_r = pool.tile([E, DSL], BF16, tag="eb1")
            nc.gpsimd.dma_start(eb1_r[:], eb1s[:])
            eb2_f = pool.tile([E, H], F32, tag="eb2f")
            nc.gpsimd.dma_start(eb2_f[:], eb2[:])
            eb2_r8 = pool.tile([E, H], BF16, tag="eb2r")
            nc.scalar.activation(eb2_r8[:], eb2_f[:], AF.Copy, scale=1.0 / NC)
            b1t = pool.tile([P, 4 * B], F32, tag="b1t")
            b2t = pool.tile([P, 8 * B], F32, tag="b2t")
            for mt in range(4):
                ps = pp.tile([P, B], F32, tag="tiny")
                nc.tensor.matmul(
                    ps[:], eb1_r[:, mt * P : (mt + 1) * P], gT_r[:],
                    start=True, stop=True,
                )
                nc.vector.tensor_copy(b1t[:, mt * B : (mt + 1) * B], ps[:])
            for m in range(8):
                ps = pp.tile([P, B], F32, tag="tiny")
                nc.tensor.matmul(
                    ps[:], eb2_r8[:, m * P : (m + 1) * P], gT_r[:],
                    start=True, stop=True,
                )
                nc.vector.tensor_copy(b2t[:, m * B : (m + 1) * B], ps[:])

            # y1 stays in SBUF: [P(dff-part), b, kt, s]
            y1 = pool.tile([P, B * 4 * S], BF16, tag="y1")
            y1v = y1[:].rearrange("p (b t s) -> p b t s", b=B, t=4)

            def gsc(b, e):
                return g_bc[:, b * E + e : b * E + e + 1]

            def make_gdiag(b):
                tiles = []
                for e in range(E):
                    gd = pool.tile([P, P], BF16, tag="gd", bufs=8)
                    nc.vector.tensor_scalar_mul(gd[:], eye_bf[:], gsc(b, e))
                    tiles.append(gd)
                return tiles

            # ---- merge helpers (w_dst view [P, nk, 512]-ish) ----
            def merge_pe(b, dstv, srcv, nkt, gds):
                """dstv: [P, nkt, FD] bf16; srcv: [P, nkt, E, FD]."""
                for half in range(0, nkt, 4):
                    n = min(4, nkt - half)
                    pss = [
                        pp.tile([P, 512], F32, tag="mm", bufs=4, name=f"mm{i}")
                        for i in range(n)
                    ]
                    for e in range(E):
                        for i in range(n):
                            nc.tensor.matmul(
                                pss[i][:], gds[e][:], srcv[:, half + i, e],
                                start=(e == 0), stop=(e == E - 1),
                            )
                    for i in range(n):
                        nc.vector.tensor_copy(dstv[:, half + i], pss[i][:])

            def merge_sv(b, dstv, srcv, nkt, mul_engine):
                """dstv: [P, nkt, 512]; srcv: [P, nkt, E, 512].
                Chunks of 4 kt (FD=2048).  All muls write scratch v-tiles;
                the first add initializes dst (keeps Act off the w-tile
                write-ordering chain, which otherwise stalls it)."""
                for half in range(0, nkt, 4):
                    n = min(4, nkt - half)
                    d = dstv[:, half : half + n]
                    s = srcv[:, half : half + n]
                    # Pool muls (e6, e7) first so the slow Pool engine works
                    # concurrently with Act; the add chain starts from their
                    # outputs so the dst first-write stays on DVE.
                    eorder = [6, 7, 0, 1, 2, 3, 4, 5] if mul_engine is act_mul \
                        else list(range(E))
                    vs = {}
                    for i, e in enumerate(eorder):
                        v = pool.tile([P, 4 * 512], BF16, tag="tv", bufs=2,
                                      name=f"v{e}")
                        vv = v[:].rearrange("p (t f) -> p t f", t=4)[:, :n]
                        mul_engine(vv, s[:, :, e], gsc(b, e))
                        vs[e] = vv
                        if i == 1:
                            nc.vector.tensor_add(d, vs[eorder[0]], vs[eorder[1]])
                        elif i > 1:
                            nc.vector.tensor_add(d, d, vv)

            def act_mul(d, s, g):
                nc.scalar.activation(d, s, AF.Copy, scale=g)

            def dve_mul(d, s, g):
                nc.vector.tensor_scalar_mul(d, s, g)

            def merge(b, mode, dstv, srcv, nkt, gds=None):
                if mode == "pe":
                    merge_pe(b, dstv, srcv, nkt, gds or make_gdiag(b))
                elif mode == "tree":
                    merge_sv(b, dstv, srcv, nkt, act_mul)
                else:
                    merge_sv(b, dstv, srcv, nkt, dve_mul)

            def load_ew1():
                # ew1 in two k-halves, e-major: [P, e, k4, 512]; per-expert
                # DMA chunks so merges start as soon as expert 0 lands.
                ew1_t = []
                for kh in range(2):
                    t = pool.tile([P, 16384], BF16, tag="B32", bufs=3, name=f"ew1_{kh}")
                    nc.sync.dma_start(
                        t[:], ew1d[:, kh * 16384 : (kh + 1) * 16384]
                    )
                    ew1_t.append(t[:].rearrange("p (e k d) -> p k e d", k=4, e=E))
                return ew1_t

            def load_ew2(h):
                t = pool.tile([P, 16384], BF16, tag="B32", bufs=3, name=f"ew2_{h}")
                nc.sync.dma_start(t[:], ew2d[h])
                return t[:].rearrange("p (e t c) -> p t e c", e=E, t=4)

            def phase1(ew1_t=None):
                if ew1_t is None:
                    ew1_t = load_ew1()

                w1ts = {}
                for b in ALLOC1_ORDER:
                    w1 = pool.tile([P, 8 * 512], BF16, tag="w1t", bufs=4, name=f"w1_{b}")
                    w1ts[b] = w1[:].rearrange("p (k d) -> p k d", k=8)

                def m1chunk(b, kh):
                    merge(b, L1_MODE[b],
                          w1ts[b][:, kh * 4 : (kh + 1) * 4], ew1_t[kh], 4)

                pe_bs = [b for b in ALLOC1_ORDER if L1_MODE[b] == "pe"]
                tree_bs = [b for b in ALLOC1_ORDER if L1_MODE[b] == "tree"]
                pure_bs = [b for b in ALLOC1_ORDER if L1_MODE[b] == "pure"]
                for b in pe_bs:
                    gds = make_gdiag(b)
                    for kh in range(2):
                        merge(b, "pe", w1ts[b][:, kh * 4 : (kh + 1) * 4],
                              ew1_t[kh], 4, gds)
                # interleave tree/pure kh-chunks so Act and DVE ping-pong
                from itertools import zip_longest
                for t, p in zip_longest(tree_bs, pure_bs):
                    for kh in range(2):
                        if t is not None:
                            m1chunk(t, kh)
                        if p is not None:
                            m1chunk(p, kh)

                for b in GEMM1_ORDER:
                    w1v = w1ts.pop(b)
                    xb = pool.tile([P, 8 * S], BF16, tag="x8", bufs=3)
                    nc.sync.dma_start(
                        xb[:].rearrange("p (k s) -> p k s", k=8),
                        xt_all.rearrange("b (k p) s -> b p k s", p=P)[b],
                    )
                    xbv = xb[:].rearrange("p (k s) -> p k s", k=8)
                    for mt in range(4):
                        ps = pp.tile([P, S], F32, tag="out", bufs=3)
                        for k in range(8):
                            nc.tensor.matmul(
                                ps[:],
                                w1v[:, k, mt * P : (mt + 1) * P],
                                xbv[:, k],
                                start=(k == 0),
                                stop=(k == 7),
                            )
                        nc.scalar.activation(
                            y1v[:, b, mt], ps[:], AF.Relu,
                            bias=b1t[:, mt * B + b : mt * B + b + 1],
                        )

            def phase2(pre0=None, with_rs=True):
                for h in range(2):
                    ew2v = pre0 if (h == 0 and pre0 is not None) else load_ew2(h)
                    w2_sb = pool.tile([P, 16384], BF16, tag="B32", bufs=3)
                    w2v = w2_sb[:].rearrange("p (b t c) -> p b t c", b=B, t=4)
                    pe2 = [b for b in MERGE2_ORDER if L2_MODE[b] == "pe"]
                    tr2 = [b for b in MERGE2_ORDER if L2_MODE[b] == "tree"]
                    pu2 = [b for b in MERGE2_ORDER if L2_MODE[b] == "pure"]
                    gds2 = {b: make_gdiag(b) for b in pe2}
                    for b in pe2:
                        merge(b, "pe", w2v[:, b], ew2v, 4, gds2[b])
                    from itertools import zip_longest as _zl
                    for t, p in _zl(tr2, pu2):
                        if t is not None:
                            merge(t, "tree", w2v[:, t], ew2v, 4)
                        if p is not None:
                            merge(p, "pure", w2v[:, p], ew2v, 4)
                    for mp in range(2):
                        for b in GEMM2_ORDER:
                            for ml in range(2):
                                mh = mp * 2 + ml
                                mg = h * 4 + mh
                                ps = pp.tile([P, S], F32, tag="out", bufs=3)
                                for kt in range(4):
                                    nc.tensor.matmul(
                                        ps[:],
                                        w2v[:, b, kt, mh * P : (mh + 1) * P],
                                        y1v[:, b, kt],
                                        start=(kt == 0),
                                        stop=(kt == 3),
                                    )
                                y2 = pool.tile([P, S], BF16, tag="y2s", bufs=2)
                                nc.scalar.activation(
                                    y2[:], ps[:], AF.Identity,
                                    bias=b2t[:, mg * B + b : mg * B + b + 1],
                                )
                                nc.sync.dma_start(rs_in[h, mp, b, ml], y2[:])
                        if with_rs:
                            nc.gpsimd.collective_compute(
                                "ReduceScatter",
                                mybir.AluOpType.add,
                                ins=[
                                    rs_in.ap()[h, mp].rearrange(
                                        "b m p s -> (b m p) s"
                                    )
                                ],
                                outs=[rs_out[h, mp]],
                                replica_groups=groups,
                            )

            ew1_pre = None if time_loop else load_ew1()
            ew2_pre = None if time_loop else load_ew2(0)
            if time_loop:
                with tc.For_i(0, time_loop, 1):
                    if time_phase in (0, 1):
                        phase1()
                    if time_phase in (0, 2):
                        phase2(with_rs=False)
                nc.sync.dma_start(y_out[0 : 2 * P], rs_in.ap()[0, 0, 0])
            else:
                phase1(ew1_pre)
                phase2(ew2_pre, with_rs=True)
                for h in range(2):
                    for mp in range(2):
                        nc.sync.dma_start(
                            y_out[(h * 4 + mp * 2) * P : (h * 4 + mp * 2 + 2) * P],
                            rs_out[h, mp],
                        )

    nc.compile()
    return nc


# ---------------- host-side sharding ----------------

def _bf16(a):
    import ml_dtypes
    return np.asarray(a, np.float32).astype(ml_dtypes.bfloat16)


def _ew1_dev(a):
    # a: [E, DSL(o=d), H(i)] -> [P, 2kh * E * 4k * DSL] with [p][kh][e][k4][d]
    a2 = np.ascontiguousarray(np.asarray(a, np.float32).transpose(2, 0, 1))
    a3 = a2.reshape(2, 4, P, E, DSL).transpose(2, 0, 3, 1, 4)
    return _bf16(np.ascontiguousarray(a3.reshape(P, 8 * E * DSL)))


def _ew2_dev(c):
    # c: [E, H(o), DSL(i)] -> [2h, P, E*4kt*512] with [h][p][e][kt][c]
    c2 = np.ascontiguousarray(np.asarray(c, np.float32).transpose(2, 0, 1))
    c3 = c2.reshape(4, P, E, 2, 512).transpose(3, 1, 2, 0, 4)
    return _bf16(np.ascontiguousarray(c3.reshape(2, P, E * 4 * 512)))


def _shard_inputs(hidden_states, gW1, gb1, gW2, gb2, eW1, eb1, eW2, eb2):
    xt_all = _bf16(
        np.ascontiguousarray(
            np.asarray(hidden_states, dtype=np.float32).transpose(1, 2, 0)
        )
    )  # [B, H, S]
    gW1t = _bf16(np.ascontiguousarray(np.asarray(gW1, np.float32).T))
    gb1t = np.ascontiguousarray(np.asarray(gb1, np.float32).reshape(8, P).T)
    gW2t = _bf16(np.ascontiguousarray(np.asarray(gW2, np.float32).T))
    gb2 = np.ascontiguousarray(np.asarray(gb2, np.float32))
    eW1 = np.asarray(eW1, np.float32)
    eW2 = np.asarray(eW2, np.float32)
    eb1 = np.asarray(eb1, np.float32)
    eb2 = np.ascontiguousarray(np.asarray(eb2, np.float32))
    in_maps = []
    for j in range(NC):
        sl = slice(j * DSL, (j + 1) * DSL)
        in_maps.append(
            {
                "xt_all": xt_all,
                "xt_own": np.ascontiguousarray(xt_all[j]),
                "gw1t": gW1t,
                "gb1t": gb1t,
                "gw2t": gW2t,
                "gb2": gb2,
                "ew1d": _ew1_dev(eW1[:, sl, :]),
                "ew2d": _ew2_dev(eW2[:, :, sl]),
                "eb1s": _bf16(np.ascontiguousarray(eb1[:, sl])),
                "eb2": eb2,
            }
        )
    return in_maps


# ---------------- SPMD runner (persistent jit over axon PJRT) -----------

_CACHE = {}


def _build_runner(time_loop=0, time_phase=0):
    import jax
    from jax.sharding import Mesh, PartitionSpec
    from jax.experimental.shard_map import shard_map
    from concourse import bass2jax

    nc = build_module(time_loop=time_loop, time_phase=time_phase)
    bass2jax.install_neuronx_cc_hook()
    partition_name = nc.partition_id_tensor.name if nc.partition_id_tensor else None

    in_names, out_names, out_avals = [], [], []
    for alloc in nc.m.functions[0].allocations:
        if not isinstance(alloc, mybir.MemoryLocationSet):
            continue
        name = alloc.memorylocations[0].name
        if alloc.kind == "ExternalInput":
            if name != partition_name:
                in_names.append(name)
        elif alloc.kind == "ExternalOutput":
            out_avals.append(
                jax.core.ShapedArray(
                    tuple(alloc.tensor_shape), mybir.dt.np(alloc.dtype)
                )
            )
            out_names.append(name)
    n_outs = len(out_names)
    all_in_names = list(in_names) + list(out_names)
    if partition_name is not None:
        all_in_names.append(partition_name)

    def _body(*args):
        operands = list(args)
        if partition_name is not None:
            operands.append(bass2jax.partition_id_tensor())
        return tuple(
            bass2jax._bass_exec_p.bind(
                *operands,
                out_avals=tuple(out_avals),
                in_names=tuple(all_in_names),
                out_names=tuple(out_names),
                lowering_input_output_aliases=(),
                sim_require_finite=True,
                sim_require_nnan=True,
                nc=nc,
            )
        )

    devices = jax.devices()[:NC]
    mesh = Mesh(np.asarray(devices), ("core",))
    n_params = len(in_names)
    sharded = jax.jit(
        shard_map(
            _body,
            mesh=mesh,
            in_specs=(PartitionSpec("core"),) * (n_params + n_outs),
            out_specs=(PartitionSpec("core"),) * n_outs,
            check_rep=False,
        ),
        keep_unused=True,
    )
    zero_shapes = [((NC * a.shape[0], *a.shape[1:]), a.dtype) for a in out_avals]

    def run(in_maps, device_inputs=None, fetch=True):
        if device_inputs is None:
            concat_in = [
                np.concatenate(
                    [np.asarray(in_maps[c][n]) for c in range(NC)], axis=0
                )
                for n in in_names
            ]
            dev_params = [jax.device_put(x) for x in concat_in]
            dev_zeros = [jax.device_put(np.zeros(s, d)) for s, d in zero_shapes]
            device_inputs = (dev_params, dev_zeros)
            jax.block_until_ready(dev_params)
            jax.block_until_ready(dev_zeros)
        dev_params, dev_zeros = device_inputs
        out_arrs = sharded(*dev_params, *dev_zeros)
        jax.block_until_ready(out_arrs)
        if not fetch:
            return None, device_inputs
        results = [
            {
                name: np.asarray(out_arrs[i]).reshape(NC, *out_avals[i].shape)[c]
                for i, name in enumerate(out_names)
            }
            for c in range(NC)
        ]
        return results, device_inputs

    return run


def get_runner(time_loop=0, time_phase=0):
    key = ("run", time_loop, time_phase)
    if key not in _CACHE:
        _CACHE[key] = _build_runner(time_loop=time_loop, time_phase=time_phase)
    return _CACHE[key]


def kernel(**inputs) -> np.ndarray:
    run = get_runner()
    in_maps = _shard_inputs(**inputs)
    results, _ = run(in_maps)
    # core b's output is y2^T[b] = [H, S] bf16; assemble [S, B, H] f32
    y2t = np.stack(
        [results[b]["y2t"].astype(np.float32) for b in range(B)], axis=0
    )  # [B, H, S]
    return np.ascontiguousarray(y2t.transpose(2, 0, 1)).astype(np.float32)


def build_collective_bench(reps):
    """Standalone module issuing `reps` x (AllGather + 4 RS chunks),
    serialized by WAR on rs_out, for timing the collective stream."""
    nc = bacc.Bacc()
    xt_all = nc.declare_dram_parameter("xt_all", [B, H, S], BF16, isOutput=False)
    gb2 = nc.declare_dram_parameter("gb2", [E], F32, isOutput=False)
    y_out = nc.declare_dram_parameter("y2t", [H, S], BF16, isOutput=True)
    ag_in = nc.dram_tensor("ag_in", [E], F32)
    ag_out = nc.dram_tensor("ag_out", [NC * E], F32, addr_space="Shared")
    rs_in = nc.dram_tensor("rs_in", [2, 2, B, 2, P, S], BF16)
    rs_out = nc.dram_tensor("rs_out", [2, 2, 2 * P, S], BF16)
    groups = [list(range(NC))]
    with TileContext(nc) as tc:  # noqa: F841
        nc.sync.dma_start(
            rs_in.ap().rearrange("a c b m p s -> (a c b m p) s"),
            xt_all.ap().rearrange("b (r p) s -> (b r p) s", p=P),
        )
        nc.sync.dma_start(ag_in[:], gb2[:])
        for _ in range(reps):
            nc.gpsimd.collective_compute(
                "AllGather", mybir.AluOpType.bypass,
                ins=[ag_in[:]], outs=[ag_out[:]], replica_groups=groups,
            )
            for h in range(2):
                for mp in range(2):
                    nc.gpsimd.collective_compute(
                        "ReduceScatter", mybir.AluOpType.add,
                        ins=[rs_in.ap()[h, mp].rearrange("b m p s -> (b m p) s")],
                        outs=[rs_out[h, mp]], replica_groups=groups,
                    )
        nc.sync.dma_start(y_out[0 : 2 * P], rs_out[0, 0])
    nc.compile()
    return nc


def get_collective_runner(reps):
    key = ("coll", reps)
    if key not in _CACHE:
        import functools
        global build_module
        orig = build_module
        try:
            build_module = functools.partial(_cb_shim, reps)
            _CACHE[key] = _build_runner(time_loop=0, time_phase=0)
        finally:
            build_module = orig
    return _CACHE[key]


def _cb_shim(reps, time_loop=0, time_phase=0):
    return build_collective_bench(reps)



# revision 11
# speedup vs baseline: 1.9487x; 1.9487x over previous
"""Trainium2 Bass kernel for nn_DictMoEDirect (moe_routing), v3.

Reference computation (shapes hardcoded):
  x = hidden_states.transpose(1,0,2)              # [B,S,H]
  g = mean_s(relu(x@gW1.T + gb1) @ gW2.T + gb2)   # [B,E]
  W1_b = sum_e g[b,e] eW1[e]; b1_b = g[b]@eb1     # per-sample merged MLP
  W2_b = sum_e g[b,e] eW2[e]; b2_b = g[b]@eb2
  y = relu(x@W1_b.T + b1_b) @ W2_b.T + b2_b       # [B,S,H]
  return y.transpose(1,0,2)                       # [S,B,H]

Distribution over 8 NeuronCores (v3):
  - Gate: data-parallel (core b computes g[b]), tiny AllGather of g.
  - FFN: tensor-parallel over DFF (core j owns a 512-wide slice).  Layer-2
    partial products are reduce-scattered in 4 bf16 chunks.
  - NEW in v3: the per-sample weight merges (W_b = sum_e g[b,e] E_e) run as
    single-pass PE matmuls with a block-diagonal g as the moving operand:
      stationary lhsT[(e + 8c), m] = E[e, d=q*16+c, i=k*128+m]   (per k,q)
      moving    rhs[(e + 8c), b*16 + c2] = g[b,e] * delta(c,c2)
      out[m, b*16 + c2] = W_b[d=q*16+c2, i=k*128+m]
    One pass consumes each expert element once and produces ALL 8 samples'
    merged weights at full PE stream rate (128 elem/cycle): 256 x 128-row
    matmuls per layer = ~13.7us, vs ~109us/layer for the diag-matmul trick.
    PSUM->SBUF drains (DVE/ACT alternating) de-interleave (b,d) so the
    per-sample GEMM lhsT slices stay contiguous.
  - All bulk data is bf16; y1 stays resident in SBUF.

kernel(**inputs) takes full unsharded inputs, shards/transposes on the host,
runs the SPMD kernel, reassembles the full [S,B,H] output.
"""

import numpy as np

import concourse.bass as bass  # noqa: F401
import concourse.mybir as mybir
from concourse import bacc
from concourse.tile import TileContext

H = 1024
DFF = 4096
E = 8
B = 8
S = 512
NC = 8
DSL = DFF // NC  # 512, per-core DFF slice
P = 128
F32 = mybir.dt.float32
BF16 = mybir.dt.bfloat16
AF = mybir.ActivationFunctionType


def build_module(time_loop=0, time_phase=0):
    """time_loop=R wraps the FFN phases (not gate/collectives) in an
    on-device For loop for timing runs; outputs are then meaningless."""
    nc = bacc.Bacc()

    # ---- I/O (all per-core) ----
    xt_all = nc.declare_dram_parameter("xt_all", [B, H, S], BF16, isOutput=False)
    xt_own = nc.declare_dram_parameter("xt_own", [H, S], BF16, isOutput=False)
    gw1t = nc.declare_dram_parameter("gw1t", [H, H], BF16, isOutput=False)
    gb1t = nc.declare_dram_parameter("gb1t", [P, 8], F32, isOutput=False)
    gw2t = nc.declare_dram_parameter("gw2t", [H, E], BF16, isOutput=False)
    gb2 = nc.declare_dram_parameter("gb2", [E], F32, isOutput=False)
    # ew1s[e + 8c, (k*32 + q)*128 + i] = eW1[e, j*512 + q*16 + c, k*128 + i]
    ew1s = nc.declare_dram_parameter("ew1s", [P, 8 * 32 * P], BF16, isOutput=False)
    # ew2s[e + 8c, (kt*64 + hg)*128 + d] = eW2[e, hg*16 + c, j*512 + kt*128 + d]
    ew2s = nc.declare_dram_parameter("ew2s", [P, 4 * 64 * P], BF16, isOutput=False)
    # gmask[e + 8c, b*16 + c2] = (c == c2)
    gmask = nc.declare_dram_parameter("gmask", [P, P], BF16, isOutput=False)
    # maskb[b'*8+e', b*16+c2] = (b' == b);  maske[b'*8+e', c*8+e] = (e' == e)
    maskb = nc.declare_dram_parameter("maskb", [8 * E, P], BF16, isOutput=False)
    maske = nc.declare_dram_parameter("maske", [8 * E, P], BF16, isOutput=False)
    eb1s = nc.declare_dram_parameter("eb1s", [E, DSL], BF16, isOutput=False)
    eb2 = nc.declare_dram_parameter("eb2", [E, H], F32, isOutput=False)
    y_out = nc.declare_dram_parameter("y2t", [H, S], BF16, isOutput=True)

    # ---- internal DRAM ----
    ag_in = nc.dram_tensor("ag_in", [E], F32)
    ag_out = nc.dram_tensor("ag_out", [NC * E], F32, addr_space="Shared")
    rs_in = nc.dram_tensor("rs_in", [2, 2, B, 2, P, S], BF16)
    rs_out = nc.dram_tensor("rs_out", [2, 2, 2 * P, S], BF16)
    groups = [list(range(NC))]

    with TileContext(nc) as tc:
        with (
            tc.tile_pool(name="main", bufs=1) as pool,
            tc.tile_pool(name="psum", bufs=1, space="PSUM") as pp,
        ):
            # =================== gate (own sample) ===================
            xo = pool.tile([P, 8 * S], BF16, tag="x8", bufs=3)
            nc.sync.dma_start(
                xo[:].rearrange("p (k s) -> p k s", k=8),
                xt_own.rearrange("(k p) s -> p k s", p=P),
            )
            gb1_sb = pool.tile([P, 8], F32, tag="gb1")
            nc.sync.dma_start(gb1_sb[:], gb1t[:])
            gw1_sb = pool.tile([P, 32768], BF16, tag="B64", bufs=2)
            gw1v = gw1_sb[:].rearrange("p (k o) -> p k o", k=8)[:, :, :H]
            for k in range(8):
                nc.sync.dma_start(gw1v[:, k], gw1t[k * P : (k + 1) * P, :])
            h1 = pool.tile([P, 8 * S], BF16, tag="x8", bufs=3)
            h1v = h1[:].rearrange("p (m s) -> p m s", m=8)
            for m in range(8):
                ps = pp.tile([P, S], F32, tag="out", bufs=3)
                for k in range(8):
                    nc.tensor.matmul(
                        ps[:],
                        gw1v[:, k, m * P : (m + 1) * P],
                        xo[:, k * S : (k + 1) * S],
                        start=(k == 0),
                        stop=(k == 7),
                    )
                nc.scalar.activation(
                    h1v[:, m], ps[:], AF.Relu, bias=gb1_sb[:, m : m + 1]
                )
            gw2_sb = pool.tile([P, 8 * E], BF16, tag="gw2")
            for k in range(8):
                nc.sync.dma_start(
                    gw2_sb[:, k * E : (k + 1) * E], gw2t[k * P : (k + 1) * P, :]
                )
            ps_g = pp.tile([E, S], F32, tag="tiny")
            for k in range(8):
                nc.tensor.matmul(
                    ps_g[:],
                    gw2_sb[:, k * E : (k + 1) * E],
                    h1v[:, k],
                    start=(k == 0),
                    stop=(k == 7),
                )
            gsum = pool.tile([E, 1], F32, tag="gsum")
            nc.vector.reduce_sum(gsum[:], ps_g[:], axis=mybir.AxisListType.X)
            gb2_sb = pool.tile([E, 1], F32, tag="gb2")
            nc.gpsimd.dma_start(gb2_sb[:], gb2[:, None])
            gmean = pool.tile([E, 1], F32, tag="gmean")
            nc.vector.tensor_scalar_mul(gmean[:], gsum[:], 1.0 / S)
            gown = pool.tile([E, 1], F32, tag="gown")
            nc.vector.tensor_add(gown[:], gmean[:], gb2_sb[:])
            nc.sync.dma_start(ag_in[:], gown[:, 0])

            nc.gpsimd.collective_compute(
                "AllGather",
                mybir.AluOpType.bypass,
                ins=[ag_in[:]],
                outs=[ag_out[:]],
                replica_groups=groups,
            )

            # ---- block-diagonal g (moving operand of all merges) ----
            # g64[b*8+e, 0] = g[b, e]; rhsg = maskb * g64 (per-part scalar);
            # garr[(c e), (b c2)] = maske.T @ rhsg = g[b, e];
            # gblk = garr * gmask  (bf16)
            gmask_sb = pool.tile([P, P], BF16, tag="gmask")
            nc.sync.dma_start(gmask_sb[:], gmask[:])
            maskb_sb = pool.tile([8 * E, P], BF16, tag="maskb")
            nc.sync.dma_start(maskb_sb[:], maskb[:])
            maske_sb = pool.tile([8 * E, P], BF16, tag="maske")
            nc.sync.dma_start(maske_sb[:], maske[:])
            g64 = pool.tile([8 * E, 1], F32, tag="g64")
            nc.gpsimd.dma_start(g64[:], ag_out[:, None])
            rhsg = pool.tile([8 * E, P], BF16, tag="rhsg")
            nc.vector.tensor_scalar_mul(rhsg[:], maskb_sb[:], g64[:, 0:1])
            garr_ps = pp.tile([P, P], F32, tag="tiny")
            nc.tensor.matmul(garr_ps[:], maske_sb[:], rhsg[:], start=True, stop=True)
            gblk = pool.tile([P, P], BF16, tag="gblk")
            nc.vector.tensor_mul(gblk[:], garr_ps[:], gmask_sb[:])

            gT_f = pool.tile([E, B], F32, tag="gTf")
            nc.gpsimd.dma_start(gT_f[:], ag_out.rearrange("(b e) -> e b", e=E))
            gT_r = pool.tile([E, B], BF16, tag="gT")
            nc.vector.tensor_copy(gT_r[:], gT_f[:])

            # ---- merged per-sample biases ----
            # b1t[:, mt*8+b] = (g[b] @ eb1s)[mt-tile]      (full value)
            # b2t[:, m*8+b]  = (g[b] @ eb2)[m-tile] / 8    (1/8: summed by RS)
            eb1_r = pool.tile([E, DSL], BF16, tag="eb1")
            nc.gpsimd.dma_start(eb1_r[:], eb1s[:])
            eb2_f = pool.tile([E, H], F32, tag="eb2f")
            nc.gpsimd.dma_start(eb2_f[:], eb2[:])
            eb2_r8 = pool.tile([E, H], BF16, tag="eb2r")
            nc.scalar.activation(eb2_r8[:], eb2_f[:], AF.Copy, scale=1.0 / NC)
            b1t = pool.tile([P, 4 * B], F32, tag="b1t")
            b2t = pool.tile([P, 8 * B], F32, tag="b2t")
            for mt in range(4):
                ps = pp.tile([P, B], F32, tag="tiny")
                nc.tensor.matmul(
                    ps[:], eb1_r[:, mt * P : (mt + 1) * P], gT_r[:],
                    start=True, stop=True,
                )
                nc.vector.tensor_copy(b1t[:, mt * B : (mt + 1) * B], ps[:])
            for m in range(8):
                ps = pp.tile([P, B], F32, tag="tiny")
                nc.tensor.matmul(
                    ps[:], eb2_r8[:, m * P : (m + 1) * P], gT_r[:],
                    start=True, stop=True,
                )
                nc.vector.tensor_copy(b2t[:, m * B : (m + 1) * B], ps[:])

            # y1 stays in SBUF: [P(dff-part), b, kt, s]
            y1 = pool.tile([P, B * 4 * S], BF16, tag="y1")
            y1v = y1[:].rearrange("p (b t s) -> p b t s", b=B, t=4)

            # ---- merge drains alternate DVE / ACT ----
            def dve_copy(d, s):
                nc.vector.tensor_copy(d, s)

            def act_copy(d, s):
                nc.scalar.activation(d, s, AF.Copy)

            DRAIN = [dve_copy, act_copy]

            def load_ew1():
                t = pool.tile([P, 32768], BF16, tag="B64", bufs=2, name="ew1")
                for k in range(8):
                    nc.sync.dma_start(
                        t[:, k * 4096 : (k + 1) * 4096],
                        ew1s[:, k * 4096 : (k + 1) * 4096],
                    )
                return t[:].rearrange("p (k q i) -> p k q i", k=8, q=32)

            def alloc_ew2():
                t = pool.tile([P, 32768], BF16, tag="B64", bufs=2, name="ew2")
                return t

            def load_ew2_chunk(t, kt):
                nc.sync.dma_start(
                    t[:, kt * 8192 : (kt + 1) * 8192],
                    ew2s[:, kt * 8192 : (kt + 1) * 8192],
                )

            def load_xb():
                xb = pool.tile([P, 8 * S], BF16, tag="x8", bufs=3)
                return xb

            def merge1(ew1v):
                """All 8 samples' merged W1 -> w1 tile [p, (k b d)], d=512."""
                w1 = pool.tile([P, 32768], BF16, tag="B64", bufs=2, name="w1")
                w1d = w1[:].rearrange("p (k b d) -> p k b d", k=8, b=8)
                ei = 0
                for k in range(8):
                    for qg in range(8):
                        mm = pp.tile([P, 512], F32, tag="mm", bufs=4)
                        for qq in range(4):
                            nc.tensor.matmul(
                                mm[:, qq * P : (qq + 1) * P],
                                ew1v[:, k, qg * 4 + qq],
                                gblk[:],
                                start=True, stop=True,
                            )
                        src = mm[:].rearrange("p (q b c) -> p b q c", q=4, b=8)
                        dst = w1d[:, k, :, qg * 64 : (qg + 1) * 64].rearrange(
                            "p b (q c) -> p b q c", q=4
                        )
                        DRAIN[ei % 2](dst, src)
                        ei += 1
                return w1d

            def merge2(ew2v):
                """All 8 samples' merged W2 -> w2 tile [p, (kt b h)], h=1024."""
                w2 = pool.tile([P, 32768], BF16, tag="B64", bufs=2, name="w2")
                w2d = w2[:].rearrange("p (kt b h) -> p kt b h", kt=4, b=8)
                ei = 1
                for kt in range(4):
                    for hgg in range(16):
                        mm = pp.tile([P, 512], F32, tag="mm", bufs=4)
                        for hh in range(4):
                            nc.tensor.matmul(
                                mm[:, hh * P : (hh + 1) * P],
                                ew2v[:, kt, hgg * 4 + hh],
                                gblk[:],
                                start=True, stop=True,
                            )
                        src = mm[:].rearrange("p (q b c) -> p b q c", q=4, b=8)
                        dst = w2d[:, kt, :, hgg * 64 : (hgg + 1) * 64].rearrange(
                            "p b (q c) -> p b q c", q=4
                        )
                        DRAIN[ei % 2](dst, src)
                        ei += 1
                return w2d

            def xb_dma(xb, b):
                nc.sync.dma_start(
                    xb[:].rearrange("p (k s) -> p k s", k=8),
                    xt_all.rearrange("b (k p) s -> b p k s", p=P)[b],
                )

            def gemm1(w1d, xbs, ew2t):
                for b in range(B):
                    if b + 2 < B:
                        xbs.append(load_xb())
                        xb_dma(xbs[b + 2], b + 2)
                    if ew2t is not None and 1 <= b <= 4:
                        load_ew2_chunk(ew2t, b - 1)
                    xbv = xbs[b][:].rearrange("p (k s) -> p k s", k=8)
                    for mt in range(4):
                        ps = pp.tile([P, S], F32, tag="out", bufs=3)
                        for k in range(8):
                            nc.tensor.matmul(
                                ps[:],
                                w1d[:, k, b, mt * P : (mt + 1) * P],
                                xbv[:, k],
                                start=(k == 0),
                                stop=(k == 7),
                            )
                        nc.scalar.activation(
                            y1v[:, b, mt], ps[:], AF.Relu,
                            bias=b1t[:, mt * B + b : mt * B + b + 1],
                        )

            def gemm2(w2d, with_rs=True):
                for cg in range(4):
                    hh, mp_ = cg // 2, cg % 2
                    for b in range(B):
                        for ml in range(2):
                            mg = cg * 2 + ml
                            ps = pp.tile([P, S], F32, tag="out", bufs=3)
                            for kt in range(4):
                                nc.tensor.matmul(
                                    ps[:],
                                    w2d[:, kt, b, mg * P : (mg + 1) * P],
                                    y1v[:, b, kt],
                                    start=(kt == 0),
                                    stop=(kt == 3),
                                )
                            y2 = pool.tile([P, S], BF16, tag="y2s", bufs=2)
                            nc.scalar.activation(
                                y2[:], ps[:], AF.Identity,
                                bias=b2t[:, mg * B + b : mg * B + b + 1],
                            )
                            nc.sync.dma_start(rs_in[hh, mp_, b, ml], y2[:])
                    if with_rs:
                        nc.gpsimd.collective_compute(
                            "ReduceScatter",
                            mybir.AluOpType.add,
                            ins=[
                                rs_in.ap()[hh, mp_].rearrange(
                                    "b m p s -> (b m p) s"
                                )
                            ],
                            outs=[rs_out[hh, mp_]],
                            replica_groups=groups,
                        )

            def body(with_rs):
                ew1v = load_ew1()
                xbs = [load_xb(), load_xb()]
                xb_dma(xbs[0], 0)
                xb_dma(xbs[1], 1)
                w1d = merge1(ew1v)
                ew2t = alloc_ew2()
                gemm1(w1d, xbs, ew2t)
                ew2v = ew2t[:].rearrange("p (kt hg d) -> p kt hg d", kt=4, hg=64)
                w2d = merge2(ew2v)
                gemm2(w2d, with_rs=with_rs)

            if time_loop:
                with tc.For_i(0, time_loop, 1):
                    body(with_rs=False)
                nc.sync.dma_start(y_out[0 : 2 * P], rs_in.ap()[0, 0, 0])
            else:
                body(with_rs=True)
                for hh in range(2):
                    for mp_ in range(2):
                        nc.sync.dma_start(
                            y_out[(hh * 4 + mp_ * 2) * P : (hh * 4 + mp_ * 2 + 2) * P],
                            rs_out[hh, mp_],
                        )

    nc.compile()
    return nc


# ---------------- host-side sharding ----------------

def _bf16(a):
    import ml_dtypes
    return np.asarray(a, np.float32).astype(ml_dtypes.bfloat16)


def _ew1_dev(a):
    # a: [E, DSL(d), H(i)] -> [P, (k q i)] with partition (c e): p = 8c + e
    # ew1s[8c + e, (k*32 + q)*128 + i'] = a[e, q*16 + c, k*128 + i']
    t = np.asarray(a, np.float32).reshape(E, 32, 16, 8, P)  # [e, q, c, k, i']
    arr = t.transpose(2, 0, 3, 1, 4)  # [c, e, k, q, i']
    return _bf16(np.ascontiguousarray(arr.reshape(P, 8 * 32 * P)))


def _ew2_dev(c):
    # c: [E, H(h), DSL(d)] -> [P, (kt hg d')] with partition (c e)
    # ew2s[8c + e, (kt*64 + hg)*128 + d'] = c[e, hg*16 + c, kt*128 + d']
    t = np.asarray(c, np.float32).reshape(E, 64, 16, 4, P)  # [e, hg, c, kt, d']
    arr = t.transpose(2, 0, 3, 1, 4)  # [c, e, kt, hg, d']
    return _bf16(np.ascontiguousarray(arr.reshape(P, 4 * 64 * P)))


def _gmask():
    m = np.zeros((P, P), np.float32)
    for c in range(16):
        m[c * 8 : (c + 1) * 8, c::16] = 1.0
    return _bf16(m)


def _maskb():
    m = np.zeros((8 * E, P), np.float32)
    for b in range(8):
        m[b * 8 : (b + 1) * 8, b * 16 : (b + 1) * 16] = 1.0
    return _bf16(m)


def _maske():
    m = np.zeros((8 * E, P), np.float32)
    for e in range(E):
        m[e::8, e::8] = 1.0
    return _bf16(m)


def _shard_inputs(hidden_states, gW1, gb1, gW2, gb2, eW1, eb1, eW2, eb2):
    xt_all = _bf16(
        np.ascontiguousarray(
            np.asarray(hidden_states, dtype=np.float32).transpose(1, 2, 0)
        )
    )  # [B, H, S]
    gW1t = _bf16(np.ascontiguousarray(np.asarray(gW1, np.float32).T))
    gb1t = np.ascontiguousarray(np.asarray(gb1, np.float32).reshape(8, P).T)
    gW2t = _bf16(np.ascontiguousarray(np.asarray(gW2, np.float32).T))
    gb2 = np.ascontiguousarray(np.asarray(gb2, np.float32))
    eW1 = np.asarray(eW1, np.float32)
    eW2 = np.asarray(eW2, np.float32)
    eb1 = np.asarray(eb1, np.float32)
    eb2 = np.ascontiguousarray(np.asarray(eb2, np.float32))
    gmask = _gmask()
    maskb_h = _maskb()
    maske_h = _maske()
    in_maps = []
    for j in range(NC):
        sl = slice(j * DSL, (j + 1) * DSL)
        in_maps.append(
            {
                "xt_all": xt_all,
                "xt_own": np.ascontiguousarray(xt_all[j]),
                "gw1t": gW1t,
                "gb1t": gb1t,
                "gw2t": gW2t,
                "gb2": gb2,
                "ew1s": _ew1_dev(eW1[:, sl, :]),
                "ew2s": _ew2_dev(eW2[:, :, sl]),
                "gmask": gmask,
                "maskb": maskb_h,
                "maske": maske_h,
                "eb1s": _bf16(np.ascontiguousarray(eb1[:, sl])),
                "eb2": eb2,
            }
        )
    return in_maps


# ---------------- SPMD runner (persistent jit over axon PJRT) -----------

_CACHE = {}


def _build_runner(time_loop=0, time_phase=0):
    import jax
    from jax.sharding import Mesh, PartitionSpec
    from jax.experimental.shard_map import shard_map
    from concourse import bass2jax

    nc = build_module(time_loop=time_loop, time_phase=time_phase)
    bass2jax.install_neuronx_cc_hook()
    partition_name = nc.partition_id_tensor.name if nc.partition_id_tensor else None

    in_names, out_names, out_avals = [], [], []
    for alloc in nc.m.functions[0].allocations:
        if not isinstance(alloc, mybir.MemoryLocationSet):
            continue
        name = alloc.memorylocations[0].name
        if alloc.kind == "ExternalInput":
            if name != partition_name:
                in_names.append(name)
        elif alloc.kind == "ExternalOutput":
            out_avals.append(
                jax.core.ShapedArray(
                    tuple(alloc.tensor_shape), mybir.dt.np(alloc.dtype)
                )
            )
            out_names.append(name)
    n_outs = len(out_names)
    all_in_names = list(in_names) + list(out_names)
    if partition_name is not None:
        all_in_names.append(partition_name)

    def _body(*args):
        operands = list(args)
        if partition_name is not None:
            operands.append(bass2jax.partition_id_tensor())
        return tuple(
            bass2jax._bass_exec_p.bind(
                *operands,
                out_avals=tuple(out_avals),
                in_names=tuple(all_in_names),
                out_names=tuple(out_names),
                lowering_input_output_aliases=(),
                sim_require_finite=True,
                sim_require_nnan=True,
                nc=nc,
            )
        )

    devices = jax.devices()[:NC]
    mesh = Mesh(np.asarray(devices), ("core",))
    n_params = len(in_names)
    sharded = jax.jit(
        shard_map(
            _body,
            mesh=mesh,
            in_specs=(PartitionSpec("core"),) * (n_params + n_outs),
            out_specs=(PartitionSpec("core"),) * n_outs,
            check_rep=False,
        ),
        keep_unused=True,
    )
    zero_shapes = [((NC * a.shape[0], *a.shape[1:]), a.dtype) for a in out_avals]

    def run(in_maps, device_inputs=None, fetch=True):
        if device_inputs is None:
            concat_in = [
                np.concatenate(
                    [np.asarray(in_maps[c][n]) for c in range(NC)], axis=0
                )
                for n in in_names
            ]
            dev_params = [jax.device_put(x) for x in concat_in]
            dev_zeros = [jax.device_put(np.zeros(s, d)) for s, d in zero_shapes]
            device_inputs = (dev_params, dev_zeros)
            jax.block_until_ready(dev_params)
            jax.block_until_ready(dev_zeros)
        dev_params, dev_zeros = device_inputs
        out_arrs = sharded(*dev_params, *dev_zeros)
        jax.block_until_ready(out_arrs)
        if not fetch:
            return None, device_inputs
        results = [
            {
                name: np.asarray(out_arrs[i]).reshape(NC, *out_avals[i].shape)[c]
                for i, name in enumerate(out_names)
            }
            for c in range(NC)
        ]
        return results, device_inputs

    return run


def get_runner(time_loop=0, time_phase=0):
    key = ("run", time_loop, time_phase)
    if key not in _CACHE:
        _CACHE[key] = _build_runner(time_loop=time_loop, time_phase=time_phase)
    return _CACHE[key]


def kernel(**inputs) -> np.ndarray:
    run = get_runner()
    in_maps = _shard_inputs(**inputs)
    results, _ = run(in_maps)
    # core b's output is y2^T[b] = [H, S] bf16; assemble [S, B, H] f32
    y2t = np.stack(
        [results[b]["y2t"].astype(np.float32) for b in range(B)], axis=0
    )  # [B, H, S]
    return np.ascontiguousarray(y2t.transpose(2, 0, 1)).astype(np.float32)


def build_collective_bench(reps):
    """Standalone module issuing `reps` x (AllGather + 4 RS chunks),
    serialized by WAR on rs_out, for timing the collective stream."""
    nc = bacc.Bacc()
    xt_all = nc.declare_dram_parameter("xt_all", [B, H, S], BF16, isOutput=False)
    gb2 = nc.declare_dram_parameter("gb2", [E], F32, isOutput=False)
    y_out = nc.declare_dram_parameter("y2t", [H, S], BF16, isOutput=True)
    ag_in = nc.dram_tensor("ag_in", [E], F32)
    ag_out = nc.dram_tensor("ag_out", [NC * E], F32, addr_space="Shared")
    rs_in = nc.dram_tensor("rs_in", [2, 2, B, 2, P, S], BF16)
    rs_out = nc.dram_tensor("rs_out", [2, 2, 2 * P, S], BF16)
    groups = [list(range(NC))]
    with TileContext(nc) as tc:  # noqa: F841
        nc.sync.dma_start(
            rs_in.ap().rearrange("a c b m p s -> (a c b m p) s"),
            xt_all.ap().rearrange("b (r p) s -> (b r p) s", p=P),
        )
        nc.sync.dma_start(ag_in[:], gb2[:])
        for _ in range(reps):
            nc.gpsimd.collective_compute(
                "AllGather", mybir.AluOpType.bypass,
                ins=[ag_in[:]], outs=[ag_out[:]], replica_groups=groups,
            )
            for h in range(2):
                for mp in range(2):
                    nc.gpsimd.collective_compute(
                        "ReduceScatter", mybir.AluOpType.add,
                        ins=[rs_in.ap()[h, mp].rearrange("b m p s -> (b m p) s")],
                        outs=[rs_out[h, mp]], replica_groups=groups,
                    )
        nc.sync.dma_start(y_out[0 : 2 * P], rs_out[0, 0])
    nc.compile()
    return nc


def get_collective_runner(reps):
    key = ("coll", reps)
    if key not in _CACHE:
        import functools
        global build_module
        orig = build_module
        try:
            build_module = functools.partial(_cb_shim, reps)
            _CACHE[key] = _build_runner(time_loop=0, time_phase=0)
        finally:
            build_module = orig
    return _CACHE[key]


def _cb_shim(reps, time_loop=0, time_phase=0):
    return build_collective_bench(reps)


# revision 15
# speedup vs baseline: 2.2360x; 1.1474x over previous
"""Trainium2 Bass kernel for nn_DictMoEDirect (moe_routing), v3.

Reference computation (shapes hardcoded):
  x = hidden_states.transpose(1,0,2)              # [B,S,H]
  g = mean_s(relu(x@gW1.T + gb1) @ gW2.T + gb2)   # [B,E]
  W1_b = sum_e g[b,e] eW1[e]; b1_b = g[b]@eb1     # per-sample merged MLP
  W2_b = sum_e g[b,e] eW2[e]; b2_b = g[b]@eb2
  y = relu(x@W1_b.T + b1_b) @ W2_b.T + b2_b       # [B,S,H]
  return y.transpose(1,0,2)                       # [S,B,H]

Distribution over 8 NeuronCores (v3):
  - Gate: data-parallel (core b computes g[b]), tiny AllGather of g.
  - FFN: tensor-parallel over DFF (core j owns a 512-wide slice).  Layer-2
    partial products are reduce-scattered in 4 bf16 chunks.
  - NEW in v3: the per-sample weight merges (W_b = sum_e g[b,e] E_e) run as
    single-pass PE matmuls with a block-diagonal g as the moving operand:
      stationary lhsT[(e + 8c), m] = E[e, d=q*16+c, i=k*128+m]   (per k,q)
      moving    rhs[(e + 8c), b*16 + c2] = g[b,e] * delta(c,c2)
      out[m, b*16 + c2] = W_b[d=q*16+c2, i=k*128+m]
    One pass consumes each expert element once and produces ALL 8 samples'
    merged weights at full PE stream rate (128 elem/cycle): 256 x 128-row
    matmuls per layer = ~13.7us, vs ~109us/layer for the diag-matmul trick.
    PSUM->SBUF drains (DVE/ACT alternating) de-interleave (b,d) so the
    per-sample GEMM lhsT slices stay contiguous.
  - All bulk data is bf16; y1 stays resident in SBUF.

kernel(**inputs) takes full unsharded inputs, shards/transposes on the host,
runs the SPMD kernel, reassembles the full [S,B,H] output.
"""

import numpy as np

import concourse.bass as bass  # noqa: F401
import concourse.mybir as mybir
from concourse import bacc
from concourse.tile import TileContext

H = 1024
DFF = 4096
E = 8
B = 8
S = 512
NC = 8
DSL = DFF // NC  # 512, per-core DFF slice
P = 128
F32 = mybir.dt.float32
BF16 = mybir.dt.bfloat16
AF = mybir.ActivationFunctionType


def build_module(time_loop=0, time_phase=0):
    """time_loop=R wraps the FFN phases (not gate/collectives) in an
    on-device For loop for timing runs; outputs are then meaningless."""
    nc = bacc.Bacc()

    # ---- I/O (all per-core) ----
    xt_all = nc.declare_dram_parameter("xt_all", [B, H, S], BF16, isOutput=False)
    xt_own = nc.declare_dram_parameter("xt_own", [H, S], BF16, isOutput=False)
    gw1t = nc.declare_dram_parameter("gw1t", [H, H], BF16, isOutput=False)
    gb1t = nc.declare_dram_parameter("gb1t", [P, 8], F32, isOutput=False)
    gw2t = nc.declare_dram_parameter("gw2t", [H, E], BF16, isOutput=False)
    gb2 = nc.declare_dram_parameter("gb2", [E], F32, isOutput=False)
    # ew1s[e + 8c, (k*32 + q)*128 + i] = eW1[e, j*512 + q*16 + c, k*128 + i]
    ew1s = nc.declare_dram_parameter("ew1s", [P, 8 * 32 * P], BF16, isOutput=False)
    # ew2s[e + 8c, (kt*64 + hg)*128 + d] = eW2[e, hg*16 + c, j*512 + kt*128 + d]
    ew2s = nc.declare_dram_parameter("ew2s", [P, 4 * 64 * P], BF16, isOutput=False)
    # gmask[e + 8c, b*16 + c2] = (c == c2)
    gmask = nc.declare_dram_parameter("gmask", [P, P], BF16, isOutput=False)
    # maskb[b'*8+e', b*16+c2] = (b' == b);  maske[b'*8+e', c*8+e] = (e' == e)
    maskb = nc.declare_dram_parameter("maskb", [8 * E, P], BF16, isOutput=False)
    maske = nc.declare_dram_parameter("maske", [8 * E, P], BF16, isOutput=False)
    eb1s = nc.declare_dram_parameter("eb1s", [E, DSL], BF16, isOutput=False)
    eb2 = nc.declare_dram_parameter("eb2", [E, H], F32, isOutput=False)
    y_out = nc.declare_dram_parameter("y2t", [H, S], BF16, isOutput=True)

    # ---- internal DRAM ----
    ag_in = nc.dram_tensor("ag_in", [E], F32)
    ag_out = nc.dram_tensor("ag_out", [NC * E], F32, addr_space="Shared")
    rs_in = nc.dram_tensor("rs_in", [2, 2, B, 2, P, S], BF16)
    rs_out = nc.dram_tensor("rs_out", [2, 2, 2 * P, S], BF16)
    groups = [list(range(NC))]

    with TileContext(nc) as tc:
        with (
            tc.tile_pool(name="main", bufs=1) as pool,
            tc.tile_pool(name="psum", bufs=1, space="PSUM") as pp,
        ):
            # =================== gate (own sample) ===================
            xo = pool.tile([P, 8 * S], BF16, tag="x8", bufs=3)
            nc.sync.dma_start(
                xo[:].rearrange("p (k s) -> p k s", k=8),
                xt_own.rearrange("(k p) s -> p k s", p=P),
            )
            gb1_sb = pool.tile([P, 8], F32, tag="gb1")
            nc.sync.dma_start(gb1_sb[:], gb1t[:])
            gw1_sb = pool.tile([P, 32768], BF16, tag="B64", bufs=2)
            gw1v = gw1_sb[:].rearrange("p (k o) -> p k o", k=8)[:, :, :H]
            for k in range(8):
                nc.sync.dma_start(gw1v[:, k], gw1t[k * P : (k + 1) * P, :])
            h1 = pool.tile([P, 8 * S], BF16, tag="x8", bufs=3)
            h1v = h1[:].rearrange("p (m s) -> p m s", m=8)
            for m in range(8):
                ps = pp.tile([P, S], F32, tag="out", bufs=3)
                for k in range(8):
                    nc.tensor.matmul(
                        ps[:],
                        gw1v[:, k, m * P : (m + 1) * P],
                        xo[:, k * S : (k + 1) * S],
                        start=(k == 0),
                        stop=(k == 7),
                    )
                nc.scalar.activation(
                    h1v[:, m], ps[:], AF.Relu, bias=gb1_sb[:, m : m + 1]
                )
            gw2_sb = pool.tile([P, 8 * E], BF16, tag="gw2")
            for k in range(8):
                nc.sync.dma_start(
                    gw2_sb[:, k * E : (k + 1) * E], gw2t[k * P : (k + 1) * P, :]
                )
            ps_g = pp.tile([E, S], F32, tag="tiny")
            for k in range(8):
                nc.tensor.matmul(
                    ps_g[:],
                    gw2_sb[:, k * E : (k + 1) * E],
                    h1v[:, k],
                    start=(k == 0),
                    stop=(k == 7),
                )
            gsum = pool.tile([E, 1], F32, tag="gsum")
            nc.vector.reduce_sum(gsum[:], ps_g[:], axis=mybir.AxisListType.X)
            gb2_sb = pool.tile([E, 1], F32, tag="gb2")
            nc.gpsimd.dma_start(gb2_sb[:], gb2[:, None])
            gmean = pool.tile([E, 1], F32, tag="gmean")
            nc.vector.tensor_scalar_mul(gmean[:], gsum[:], 1.0 / S)
            gown = pool.tile([E, 1], F32, tag="gown")
            nc.vector.tensor_add(gown[:], gmean[:], gb2_sb[:])
            nc.sync.dma_start(ag_in[:], gown[:, 0])

            nc.gpsimd.collective_compute(
                "AllGather",
                mybir.AluOpType.bypass,
                ins=[ag_in[:]],
                outs=[ag_out[:]],
                replica_groups=groups,
            )

            # ---- block-diagonal g (moving operand of all merges) ----
            # g64[b*8+e, 0] = g[b, e]; rhsg = maskb * g64 (per-part scalar);
            # garr[(c e), (b c2)] = maske.T @ rhsg = g[b, e];
            # gblk = garr * gmask  (bf16)
            gmask_sb = pool.tile([P, P], BF16, tag="gmask")
            nc.sync.dma_start(gmask_sb[:], gmask[:])
            maskb_sb = pool.tile([8 * E, P], BF16, tag="maskb")
            nc.sync.dma_start(maskb_sb[:], maskb[:])
            maske_sb = pool.tile([8 * E, P], BF16, tag="maske")
            nc.sync.dma_start(maske_sb[:], maske[:])
            g64 = pool.tile([8 * E, 1], F32, tag="g64")
            nc.gpsimd.dma_start(g64[:], ag_out[:, None])
            rhsg = pool.tile([8 * E, P], BF16, tag="rhsg")
            nc.vector.tensor_scalar_mul(rhsg[:], maskb_sb[:], g64[:, 0:1])
            garr_ps = pp.tile([P, P], F32, tag="tiny")
            nc.tensor.matmul(garr_ps[:], maske_sb[:], rhsg[:], start=True, stop=True)
            gblk = pool.tile([P, P], BF16, tag="gblk")
            nc.vector.tensor_mul(gblk[:], garr_ps[:], gmask_sb[:])

            gT_f = pool.tile([E, B], F32, tag="gTf")
            nc.gpsimd.dma_start(gT_f[:], ag_out.rearrange("(b e) -> e b", e=E))
            gT_r = pool.tile([E, B], BF16, tag="gT")
            nc.vector.tensor_copy(gT_r[:], gT_f[:])

            # ---- merged per-sample biases ----
            # b1t[:, mt*8+b] = (g[b] @ eb1s)[mt-tile]      (full value)
            # b2t[:, m*8+b]  = (g[b] @ eb2)[m-tile] / 8    (1/8: summed by RS)
            eb1_r = pool.tile([E, DSL], BF16, tag="eb1")
            nc.gpsimd.dma_start(eb1_r[:], eb1s[:])
            eb2_f = pool.tile([E, H], F32, tag="eb2f")
            nc.gpsimd.dma_start(eb2_f[:], eb2[:])
            eb2_r8 = pool.tile([E, H], BF16, tag="eb2r")
            nc.scalar.activation(eb2_r8[:], eb2_f[:], AF.Copy, scale=1.0 / NC)
            b1t = pool.tile([P, 4 * B], F32, tag="b1t")
            b2t = pool.tile([P, 8 * B], F32, tag="b2t")
            for mt in range(4):
                ps = pp.tile([P, B], F32, tag="tiny")
                nc.tensor.matmul(
                    ps[:], eb1_r[:, mt * P : (mt + 1) * P], gT_r[:],
                    start=True, stop=True,
                )
                nc.vector.tensor_copy(b1t[:, mt * B : (mt + 1) * B], ps[:])
            for m in range(8):
                ps = pp.tile([P, B], F32, tag="tiny")
                nc.tensor.matmul(
                    ps[:], eb2_r8[:, m * P : (m + 1) * P], gT_r[:],
                    start=True, stop=True,
                )
                nc.vector.tensor_copy(b2t[:, m * B : (m + 1) * B], ps[:])

            # y1 stays in SBUF: [P(dff-part), b, kt, s]
            y1 = pool.tile([P, B * 4 * S], BF16, tag="y1")
            y1v = y1[:].rearrange("p (b t s) -> p b t s", b=B, t=4)

            # ---- merge drains alternate DVE / ACT ----
            def dve_copy(d, s):
                nc.vector.tensor_copy(d, s)

            def act_copy(d, s):
                nc.scalar.activation(d, s, AF.Copy)

            DRAIN = [dve_copy, act_copy]

            def load_ew1():
                t = pool.tile([P, 32768], BF16, tag="B64", bufs=2, name="ew1")
                for k in range(8):
                    nc.sync.dma_start(
                        t[:, k * 4096 : (k + 1) * 4096],
                        ew1s[:, k * 4096 : (k + 1) * 4096],
                    )
                return t[:].rearrange("p (k q i) -> p k q i", k=8, q=32)

            def alloc_ew2():
                t = pool.tile([P, 32768], BF16, tag="B64", bufs=2, name="ew2")
                return t

            def load_ew2_chunk(t, kt):
                nc.sync.dma_start(
                    t[:, kt * 8192 : (kt + 1) * 8192],
                    ew2s[:, kt * 8192 : (kt + 1) * 8192],
                )

            def load_xb():
                xb = pool.tile([P, 8 * S], BF16, tag="x8", bufs=3)
                return xb

            def merge1(ew1v):
                """All 8 samples' merged W1 -> w1 tile [p, (k b d)], d=512."""
                w1 = pool.tile([P, 32768], BF16, tag="B64", bufs=2, name="w1")
                w1d = w1[:].rearrange("p (k b d) -> p k b d", k=8, b=8)
                ei = 0
                for k in range(8):
                    for qg in range(4):
                        mm = pp.tile([P, 1024], F32, tag="mm", bufs=2)
                        for qq in range(8):
                            nc.tensor.matmul(
                                mm[:, qq * P : (qq + 1) * P],
                                ew1v[:, k, qg * 8 + qq],
                                gblk[:],
                                start=True, stop=True,
                            )
                        src = mm[:].rearrange("p (q b c) -> p b q c", q=8, b=8)
                        dst = w1d[:, k, :, qg * 128 : (qg + 1) * 128].rearrange(
                            "p b (q c) -> p b q c", q=8
                        )
                        DRAIN[ei % 2](dst, src)
                        ei += 1
                return w1d

            def merge2(ew2v):
                """All 8 samples' merged W2 -> w2 tile [p, (kt b h)], h=1024."""
                w2 = pool.tile([P, 32768], BF16, tag="B64", bufs=2, name="w2")
                w2d = w2[:].rearrange("p (kt b h) -> p kt b h", kt=4, b=8)
                ei = 1
                for kt in range(4):
                    for hgg in range(8):
                        mm = pp.tile([P, 1024], F32, tag="mm", bufs=2)
                        for hh in range(8):
                            nc.tensor.matmul(
                                mm[:, hh * P : (hh + 1) * P],
                                ew2v[:, kt, hgg * 8 + hh],
                                gblk[:],
                                start=True, stop=True,
                            )
                        src = mm[:].rearrange("p (q b c) -> p b q c", q=8, b=8)
                        dst = w2d[:, kt, :, hgg * 128 : (hgg + 1) * 128].rearrange(
                            "p b (q c) -> p b q c", q=8
                        )
                        DRAIN[ei % 2](dst, src)
                        ei += 1
                return w2d

            def xb_dma(xb, b):
                nc.sync.dma_start(
                    xb[:].rearrange("p (k s) -> p k s", k=8),
                    xt_all.rearrange("b (k p) s -> b p k s", p=P)[b],
                )

            def gemm1(w1d, xbs, ew2t):
                for b in range(B):
                    if b + 2 < B:
                        xbs.append(load_xb())
                        xb_dma(xbs[b + 2], b + 2)
                    if ew2t is not None and 3 <= b <= 6:
                        load_ew2_chunk(ew2t, b - 3)
                    xbv = xbs[b][:].rearrange("p (k s) -> p k s", k=8)
                    for mt in range(4):
                        ps = pp.tile([P, S], F32, tag="out", bufs=3)
                        for k in range(8):
                            nc.tensor.matmul(
                                ps[:],
                                w1d[:, k, b, mt * P : (mt + 1) * P],
                                xbv[:, k],
                                start=(k == 0),
                                stop=(k == 7),
                            )
                        nc.scalar.activation(
                            y1v[:, b, mt], ps[:], AF.Relu,
                            bias=b1t[:, mt * B + b : mt * B + b + 1],
                        )

            def gemm2(w2d, with_rs=True):
                for cg in range(4):
                    hh, mp_ = cg // 2, cg % 2
                    for b in range(B):
                        for ml in range(2):
                            mg = cg * 2 + ml
                            ps = pp.tile([P, S], F32, tag="out", bufs=3)
                            for kt in range(4):
                                nc.tensor.matmul(
                                    ps[:],
                                    w2d[:, kt, b, mg * P : (mg + 1) * P],
                                    y1v[:, b, kt],
                                    start=(kt == 0),
                                    stop=(kt == 3),
                                )
                            y2 = pool.tile([P, S], BF16, tag="y2s", bufs=8)
                            nc.scalar.activation(
                                y2[:], ps[:], AF.Identity,
                                bias=b2t[:, mg * B + b : mg * B + b + 1],
                            )
                            nc.sync.dma_start(rs_in[hh, mp_, b, ml], y2[:])
                    if with_rs:
                        nc.gpsimd.collective_compute(
                            "ReduceScatter",
                            mybir.AluOpType.add,
                            ins=[
                                rs_in.ap()[hh, mp_].rearrange(
                                    "b m p s -> (b m p) s"
                                )
                            ],
                            outs=[rs_out[hh, mp_]],
                            replica_groups=groups,
                        )

            def body(with_rs):
                ew1v = load_ew1()
                xbs = [load_xb(), load_xb()]
                xb_dma(xbs[0], 0)
                xb_dma(xbs[1], 1)
                w1d = merge1(ew1v)
                ew2t = alloc_ew2()
                gemm1(w1d, xbs, ew2t)
                ew2v = ew2t[:].rearrange("p (kt hg d) -> p kt hg d", kt=4, hg=64)
                w2d = merge2(ew2v)
                gemm2(w2d, with_rs=with_rs)

            if time_loop:
                with tc.For_i(0, time_loop, 1):
                    body(with_rs=False)
                nc.sync.dma_start(y_out[0 : 2 * P], rs_in.ap()[0, 0, 0])
            else:
                body(with_rs=True)
                for hh in range(2):
                    for mp_ in range(2):
                        nc.sync.dma_start(
                            y_out[(hh * 4 + mp_ * 2) * P : (hh * 4 + mp_ * 2 + 2) * P],
                            rs_out[hh, mp_],
                        )

    nc.compile()
    return nc


# ---------------- host-side sharding ----------------

def _bf16(a):
    import ml_dtypes
    return np.asarray(a, np.float32).astype(ml_dtypes.bfloat16)


def _ew1_dev(a):
    # a: [E, DSL(d), H(i)] -> [P, (k q i)] with partition (c e): p = 8c + e
    # ew1s[8c + e, (k*32 + q)*128 + i'] = a[e, q*16 + c, k*128 + i']
    t = np.asarray(a, np.float32).reshape(E, 32, 16, 8, P)  # [e, q, c, k, i']
    arr = t.transpose(2, 0, 3, 1, 4)  # [c, e, k, q, i']
    return _bf16(np.ascontiguousarray(arr.reshape(P, 8 * 32 * P)))


def _ew2_dev(c):
    # c: [E, H(h), DSL(d)] -> [P, (kt hg d')] with partition (c e)
    # ew2s[8c + e, (kt*64 + hg)*128 + d'] = c[e, hg*16 + c, kt*128 + d']
    t = np.asarray(c, np.float32).reshape(E, 64, 16, 4, P)  # [e, hg, c, kt, d']
    arr = t.transpose(2, 0, 3, 1, 4)  # [c, e, kt, hg, d']
    return _bf16(np.ascontiguousarray(arr.reshape(P, 4 * 64 * P)))


def _gmask():
    m = np.zeros((P, P), np.float32)
    for c in range(16):
        m[c * 8 : (c + 1) * 8, c::16] = 1.0
    return _bf16(m)


def _maskb():
    m = np.zeros((8 * E, P), np.float32)
    for b in range(8):
        m[b * 8 : (b + 1) * 8, b * 16 : (b + 1) * 16] = 1.0
    return _bf16(m)


def _maske():
    m = np.zeros((8 * E, P), np.float32)
    for e in range(E):
        m[e::8, e::8] = 1.0
    return _bf16(m)


def _shard_inputs(hidden_states, gW1, gb1, gW2, gb2, eW1, eb1, eW2, eb2):
    xt_all = _bf16(
        np.ascontiguousarray(
            np.asarray(hidden_states, dtype=np.float32).transpose(1, 2, 0)
        )
    )  # [B, H, S]
    gW1t = _bf16(np.ascontiguousarray(np.asarray(gW1, np.float32).T))
    gb1t = np.ascontiguousarray(np.asarray(gb1, np.float32).reshape(8, P).T)
    gW2t = _bf16(np.ascontiguousarray(np.asarray(gW2, np.float32).T))
    gb2 = np.ascontiguousarray(np.asarray(gb2, np.float32))
    eW1 = np.asarray(eW1, np.float32)
    eW2 = np.asarray(eW2, np.float32)
    eb1 = np.asarray(eb1, np.float32)
    eb2 = np.ascontiguousarray(np.asarray(eb2, np.float32))
    gmask = _gmask()
    maskb_h = _maskb()
    maske_h = _maske()
    in_maps = []
    for j in range(NC):
        sl = slice(j * DSL, (j + 1) * DSL)
        in_maps.append(
            {
                "xt_all": xt_all,
                "xt_own": np.ascontiguousarray(xt_all[j]),
                "gw1t": gW1t,
                "gb1t": gb1t,
                "gw2t": gW2t,
                "gb2": gb2,
                "ew1s": _ew1_dev(eW1[:, sl, :]),
                "ew2s": _ew2_dev(eW2[:, :, sl]),
                "gmask": gmask,
                "maskb": maskb_h,
                "maske": maske_h,
                "eb1s": _bf16(np.ascontiguousarray(eb1[:, sl])),
                "eb2": eb2,
            }
        )
    return in_maps


# ---------------- SPMD runner (persistent jit over axon PJRT) -----------

_CACHE = {}


def _build_runner(time_loop=0, time_phase=0):
    import jax
    from jax.sharding import Mesh, PartitionSpec
    from jax.experimental.shard_map import shard_map
    from concourse import bass2jax

    nc = build_module(time_loop=time_loop, time_phase=time_phase)
    bass2jax.install_neuronx_cc_hook()
    partition_name = nc.partition_id_tensor.name if nc.partition_id_tensor else None

    in_names, out_names, out_avals = [], [], []
    for alloc in nc.m.functions[0].allocations:
        if not isinstance(alloc, mybir.MemoryLocationSet):
            continue
        name = alloc.memorylocations[0].name
        if alloc.kind == "ExternalInput":
            if name != partition_name:
                in_names.append(name)
        elif alloc.kind == "ExternalOutput":
            out_avals.append(
                jax.core.ShapedArray(
                    tuple(alloc.tensor_shape), mybir.dt.np(alloc.dtype)
                )
            )
            out_names.append(name)
    n_outs = len(out_names)
    all_in_names = list(in_names) + list(out_names)
    if partition_name is not None:
        all_in_names.append(partition_name)

    def _body(*args):
        operands = list(args)
        if partition_name is not None:
            operands.append(bass2jax.partition_id_tensor())
        return tuple(
            bass2jax._bass_exec_p.bind(
                *operands,
                out_avals=tuple(out_avals),
                in_names=tuple(all_in_names),
                out_names=tuple(out_names),
                lowering_input_output_aliases=(),
                sim_require_finite=True,
                sim_require_nnan=True,
                nc=nc,
            )
        )

    devices = jax.devices()[:NC]
    mesh = Mesh(np.asarray(devices), ("core",))
    n_params = len(in_names)
    sharded = jax.jit(
        shard_map(
            _body,
            mesh=mesh,
            in_specs=(PartitionSpec("core"),) * (n_params + n_outs),
            out_specs=(PartitionSpec("core"),) * n_outs,
            check_rep=False,
        ),
        keep_unused=True,
    )
    zero_shapes = [((NC * a.shape[0], *a.shape[1:]), a.dtype) for a in out_avals]

    def run(in_maps, device_inputs=None, fetch=True):
        if device_inputs is None:
            concat_in = [
                np.concatenate(
                    [np.asarray(in_maps[c][n]) for c in range(NC)], axis=0
                )
                for n in in_names
            ]
            dev_params = [jax.device_put(x) for x in concat_in]
            dev_zeros = [jax.device_put(np.zeros(s, d)) for s, d in zero_shapes]
            device_inputs = (dev_params, dev_zeros)
            jax.block_until_ready(dev_params)
            jax.block_until_ready(dev_zeros)
        dev_params, dev_zeros = device_inputs
        out_arrs = sharded(*dev_params, *dev_zeros)
        jax.block_until_ready(out_arrs)
        if not fetch:
            return None, device_inputs
        results = [
            {
                name: np.asarray(out_arrs[i]).reshape(NC, *out_avals[i].shape)[c]
                for i, name in enumerate(out_names)
            }
            for c in range(NC)
        ]
        return results, device_inputs

    return run


def get_runner(time_loop=0, time_phase=0):
    key = ("run", time_loop, time_phase)
    if key not in _CACHE:
        _CACHE[key] = _build_runner(time_loop=time_loop, time_phase=time_phase)
    return _CACHE[key]


def kernel(**inputs) -> np.ndarray:
    run = get_runner()
    in_maps = _shard_inputs(**inputs)
    results, _ = run(in_maps)
    # core b's output is y2^T[b] = [H, S] bf16; assemble [S, B, H] f32
    y2t = np.stack(
        [results[b]["y2t"].astype(np.float32) for b in range(B)], axis=0
    )  # [B, H, S]
    return np.ascontiguousarray(y2t.transpose(2, 0, 1)).astype(np.float32)


def build_collective_bench(reps):
    """Standalone module issuing `reps` x (AllGather + 4 RS chunks),
    serialized by WAR on rs_out, for timing the collective stream."""
    nc = bacc.Bacc()
    xt_all = nc.declare_dram_parameter("xt_all", [B, H, S], BF16, isOutput=False)
    gb2 = nc.declare_dram_parameter("gb2", [E], F32, isOutput=False)
    y_out = nc.declare_dram_parameter("y2t", [H, S], BF16, isOutput=True)
    ag_in = nc.dram_tensor("ag_in", [E], F32)
    ag_out = nc.dram_tensor("ag_out", [NC * E], F32, addr_space="Shared")
    rs_in = nc.dram_tensor("rs_in", [2, 2, B, 2, P, S], BF16)
    rs_out = nc.dram_tensor("rs_out", [2, 2, 2 * P, S], BF16)
    groups = [list(range(NC))]
    with TileContext(nc) as tc:  # noqa: F841
        nc.sync.dma_start(
            rs_in.ap().rearrange("a c b m p s -> (a c b m p) s"),
            xt_all.ap().rearrange("b (r p) s -> (b r p) s", p=P),
        )
        nc.sync.dma_start(ag_in[:], gb2[:])
        for _ in range(reps):
            nc.gpsimd.collective_compute(
                "AllGather", mybir.AluOpType.bypass,
                ins=[ag_in[:]], outs=[ag_out[:]], replica_groups=groups,
            )
            for h in range(2):
                for mp in range(2):
                    nc.gpsimd.collective_compute(
                        "ReduceScatter", mybir.AluOpType.add,
                        ins=[rs_in.ap()[h, mp].rearrange("b m p s -> (b m p) s")],
                        outs=[rs_out[h, mp]], replica_groups=groups,
                    )
        nc.sync.dma_start(y_out[0 : 2 * P], rs_out[0, 0])
    nc.compile()
    return nc


def get_collective_runner(reps):
    key = ("coll", reps)
    if key not in _CACHE:
        import functools
        global build_module
        orig = build_module
        try:
            build_module = functools.partial(_cb_shim, reps)
            _CACHE[key] = _build_runner(time_loop=0, time_phase=0)
        finally:
            build_module = orig
    return _CACHE[key]


def _cb_shim(reps, time_loop=0, time_phase=0):
    return build_collective_bench(reps)
